# revision 9
# baseline (speedup 1.0000x reference)
"""Trainium2 Bass kernel for nn_AnteLayer (fuzzy-rule antecedents over graph edges).

Per edge e: x1 = feat[dst,0]-feat[src,0], x2 = feat[dst,1]-feat[src,1],
ante[e, 3j+k] = exp(-2*(x1-c_j)^2) * exp(-2*(x2-c_k)^2),  c in {-1, 0, 1}.

Distribution: edge-parallel across 8 NeuronCores (800K edges each). The host
stages the per-edge coordinate deltas (x1/x2 planes, fp16); the device
streams, per tile:
  DMA-in x block -> 3x Derivative_Erf (ACT, one op per center over both
  planes) -> 9 rule products as 3 broadcast tensor_tensor ops (DVE) ->
  fp16 DMA-out on the two HWDGE queues.

Both DMA directions use tile-blocked DRAM layouts so every (partition, tile)
transfer is ONE contiguous run on both the SBUF and DRAM side -- one DMA
descriptor per partition instead of 9 (descriptor processing, at ~100ns each
across 16 SDMA engines, was the previous bottleneck). The host lays input
tiles as [plane0 | plane1] blocks and re-interleaves the rule-major output.

exp(-2(x-c)^2) == (sqrt(pi)/2) * Derivative_Erf(sqrt(2)*x - sqrt(2)*c); the
device emits D1*D2 = (4/pi)*ante in fp16 and the host folds the constant
pi/4 into the fp16->fp32 widening pass (a global scale, like the fp16
encoding itself). Tiles are size-graded (small first/last) so the ACT->DVE->
DMA pipeline fills fast and drains with a short tail; the ACT spline table
is preloaded via a dummy activation that overlaps the input DMA.
"""
import sys

for _p in ("/opt/trn_rl_repo", "/opt/pypackages"):
    if _p not in sys.path:
        sys.path.insert(0, _p)

import math
import numpy as np

import concourse.bass as bass
import concourse.mybir as mybir
from concourse import bacc, tile
from concourse.bass_utils import run_bass_kernel_spmd

N_CORES = 8
N_EDGES = 6400000
P = 128                       # SBUF partitions
E_CORE = N_EDGES // N_CORES   # 800000 edges per core
R = E_CORE // P               # 6250 edges per partition
TILE_SIZES = (250, 500, 1250, 1250, 1250, 1250, 500)
TMAX = max(TILE_SIZES)
assert sum(TILE_SIZES) == R

MF_CENTERS = (-1.0, 0.0, 1.0)
SQRT2 = math.sqrt(2.0)
PI_4 = math.pi / 4.0

_nc_cache = {}


def _build():
    if "nc" in _nc_cache:
        return _nc_cache["nc"]
    nc = bacc.Bacc("TRN2", target_bir_lowering=False)
    f32 = mybir.dt.float32
    f16 = mybir.dt.float16
    # tile-blocked input: per tile a [P, 2*ts] block = [x1 plane | x2 plane]
    x_ext = nc.declare_dram_parameter("xy", [P, 2 * R], f16, isOutput=False)
    # tile-blocked output: per tile a [P, 9*ts] block, rule-major inside
    out_ext = nc.declare_dram_parameter("out", [P, 9 * R], f16, isOutput=True)

    with tile.TileContext(nc) as tc:
        with (
            tc.tile_pool(name="consts", bufs=1) as consts,
            tc.tile_pool(name="xall", bufs=1) as xall,
            tc.tile_pool(name="mid", bufs=3) as mid,
            tc.tile_pool(name="oute", bufs=3) as oute,
        ):
            bias_aps = []
            for ci, c in enumerate(MF_CENTERS):
                b = consts.tile([P, 1], f32, tag=f"bias{ci}")
                nc.vector.memset(b[:, :], -SQRT2 * c)
                bias_aps.append(b)
            # Preload the ACT spline table set (Derivative_Erf) with a dummy
            # op so the table DMA overlaps the input prefetch.
            warm = consts.tile([P, 2], f16, tag="warm")
            nc.scalar.activation(
                warm[:, 1:2], warm[:, 0:1],
                mybir.ActivationFunctionType.Derivative_Erf,
                bias=bias_aps[1][:, :], scale=SQRT2,
            )
            # Phase A: prefetch ALL inputs (3.2MB fp16); tile 0 lands first.
            x_tiles = []
            t0 = 0
            for ti, ts in enumerate(TILE_SIZES):
                x = xall.tile([P, 2, ts], f16, tag=f"x{ti}")
                eng = nc.sync if ti % 2 == 0 else nc.gpsimd
                eng.dma_start(
                    out=x[:, :, :], in_=x_ext[:, 2 * t0:2 * t0 + 2 * ts])
                x_tiles.append(x)
                t0 += ts

            # Phase B: compute + output stream
            t0 = 0
            for ti, ts in enumerate(TILE_SIZES):
                x = x_tiles[ti]

                # D[p,c,m,:ts] = Derivative_Erf(sqrt2*X - sqrt2*center_c),
                # one [P, 2*ts] op per center (fixed TMAX pitch, :ts slice)
                d = mid.tile([P, 3, 2, TMAX], f16, tag="d")
                for ci in range(3):
                    nc.scalar.activation(
                        d[:, ci, :, :ts],
                        x[:, :, :],
                        mybir.ActivationFunctionType.Derivative_Erf,
                        bias=bias_aps[ci][:, :],
                        scale=SQRT2,
                    )

                # ante[p,3j+k,:] = D[p,j,0,:] * D[p,k,1,:] -- 3 tensor_tensor
                # ops, j-plane broadcast via stride-0 middle dim. Exact-size
                # tile so each j-chunk is one contiguous per-partition run;
                # each chunk's DMA fires as soon as its product op is done,
                # smoothing the output stream. Buffers reused by size.
                ante = oute.tile([P, 9, ts], f16, tag=f"ante{ts}")
                dy = d[:, :, 1, :ts]
                for j in range(3):
                    dx_ap = bass.AP(
                        d.tensor, d.offset + j * 2 * TMAX,
                        [[6 * TMAX, P], [0, 3], [1, ts]],
                    )
                    nc.vector.tensor_tensor(
                        ante[:, 3 * j:3 * j + 3, :], dx_ap, dy,
                        op=mybir.AluOpType.mult,
                    )
                    eng = nc.sync if (3 * ti + j) % 2 == 0 else nc.scalar
                    eng.dma_start(
                        out=out_ext[:, 9 * t0 + 3 * j * ts:
                                    9 * t0 + 3 * (j + 1) * ts],
                        in_=ante[:, 3 * j:3 * j + 3, :])
                t0 += ts

    nc.compile()
    _nc_cache["nc"] = nc
    return nc


def _shard_host(feat2, src_shard, dst_shard):
    # [P, 2*R] tile-blocked per-edge coordinate deltas, fp16 on the wire
    g = (feat2[dst_shard] - feat2[src_shard]).astype(np.float16)  # [E_CORE, 2]
    g = g.reshape(P, R, 2)
    blocks = []
    t0 = 0
    for ts in TILE_SIZES:
        # [P, 2, ts]: plane-separated within the tile block
        blocks.append(g[:, t0:t0 + ts, :].transpose(0, 2, 1).reshape(P, 2 * ts))
        t0 += ts
    return np.ascontiguousarray(np.concatenate(blocks, axis=1))


def kernel(feat, edge_src, edge_dst, etypes):
    feat = np.asarray(feat, dtype=np.float32)
    edge_src = np.asarray(edge_src, dtype=np.int32)
    edge_dst = np.asarray(edge_dst, dtype=np.int32)
    del etypes  # unused by the reference computation

    nc = _build()

    feat2 = np.ascontiguousarray(feat[:, :2])  # only coords participate
    in_maps = []
    for c in range(N_CORES):
        sl = slice(c * E_CORE, (c + 1) * E_CORE)
        in_maps.append({
            "xy": _shard_host(feat2, edge_src[sl], edge_dst[sl]),
        })

    res = run_bass_kernel_spmd(nc, in_maps, core_ids=list(range(N_CORES)))
    out = np.empty((N_EDGES, 9), dtype=np.float32)
    scale = np.float32(PI_4)
    for c in range(N_CORES):
        r = res.results[c]["out"]          # [P, 9*R] fp16, (4/pi)*ante
        ov = out[c * E_CORE:(c + 1) * E_CORE].reshape(P, R, 9)
        t0 = 0
        for ts in TILE_SIZES:
            blk = r[:, 9 * t0:9 * t0 + 9 * ts].reshape(P, 9, ts)
            np.multiply(blk.transpose(0, 2, 1), scale, out=ov[:, t0:t0 + ts, :])
            t0 += ts
    return out


# revision 11
# speedup vs baseline: 1.1030x; 1.1030x over previous
"""Trainium2 Bass kernel for nn_AnteLayer (fuzzy-rule antecedents over graph edges).

Per edge e: x1 = feat[dst,0]-feat[src,0], x2 = feat[dst,1]-feat[src,1],
ante[e, 3j+k] = exp(-2*(x1-c_j)^2) * exp(-2*(x2-c_k)^2),  c in {-1, 0, 1}.

Distribution: edge-parallel across 8 NeuronCores (800K edges each). The host
stages the per-edge coordinate deltas (x1/x2 planes, fp16); the device
streams, per tile:
  DMA-in x block -> 3x Derivative_Erf (ACT, one op per center over both
  planes) -> 9 rule products as 3 broadcast tensor_tensor ops (DVE) ->
  fp16 DMA-out on the two HWDGE queues.

Both DMA directions use tile-blocked DRAM layouts so every (partition, tile)
transfer is ONE contiguous run on both the SBUF and DRAM side -- one DMA
descriptor per partition instead of 9 (descriptor processing, at ~100ns each
across 16 SDMA engines, was the previous bottleneck). The host lays input
tiles as [plane0 | plane1] blocks and re-interleaves the rule-major output.

exp(-2(x-c)^2) == (sqrt(pi)/2) * Derivative_Erf(sqrt(2)*x - sqrt(2)*c); the
device emits D1*D2 = (4/pi)*ante in fp16 and the host folds the constant
pi/4 into the fp16->fp32 widening pass (a global scale, like the fp16
encoding itself). Tiles are size-graded (small first/last) so the ACT->DVE->
DMA pipeline fills fast and drains with a short tail; the ACT spline table
is preloaded via a dummy activation that overlaps the input DMA.
"""
import sys

for _p in ("/opt/trn_rl_repo", "/opt/pypackages"):
    if _p not in sys.path:
        sys.path.insert(0, _p)

import math
import numpy as np

import concourse.bass as bass
import concourse.mybir as mybir
from concourse import bacc, tile
from concourse.bass_utils import run_bass_kernel_spmd

N_CORES = 8
N_EDGES = 6400000
P = 128                       # SBUF partitions
E_CORE = N_EDGES // N_CORES   # 800000 edges per core
R = E_CORE // P               # 6250 edges per partition
TILE_SIZES = (250, 750, 1250, 1250, 1250, 750, 500, 250)
TMAX = max(TILE_SIZES)
assert sum(TILE_SIZES) == R

MF_CENTERS = (-1.0, 0.0, 1.0)
SQRT2 = math.sqrt(2.0)
PI_4 = math.pi / 4.0

_nc_cache = {}


def _build():
    if "nc" in _nc_cache:
        return _nc_cache["nc"]
    nc = bacc.Bacc("TRN2", target_bir_lowering=False)
    f32 = mybir.dt.float32
    f16 = mybir.dt.float16
    # tile-blocked input: per tile a [P, 2*ts] block = [x1 plane | x2 plane]
    x_ext = nc.declare_dram_parameter("xy", [P, 2 * R], f16, isOutput=False)
    # tile-blocked output: per tile a [P, 9*ts] block, rule-major inside
    out_ext = nc.declare_dram_parameter("out", [P, 9 * R], f16, isOutput=True)

    with tile.TileContext(nc) as tc:
        with (
            tc.tile_pool(name="consts", bufs=1) as consts,
            tc.tile_pool(name="xall", bufs=1) as xall,
            tc.tile_pool(name="mid", bufs=3) as mid,
            tc.tile_pool(name="oute", bufs=2) as oute,
        ):
            bias_aps = []
            for ci, c in enumerate(MF_CENTERS):
                b = consts.tile([P, 1], f32, tag=f"bias{ci}")
                nc.vector.memset(b[:, :], -SQRT2 * c)
                bias_aps.append(b)
            # Preload the ACT spline table set (Derivative_Erf) with a dummy
            # op so the table DMA overlaps the input prefetch.
            warm = consts.tile([P, 2], f16, tag="warm")
            nc.scalar.activation(
                warm[:, 1:2], warm[:, 0:1],
                mybir.ActivationFunctionType.Derivative_Erf,
                bias=bias_aps[1][:, :], scale=SQRT2,
            )
            # Phase A: prefetch ALL inputs (3.2MB fp16); tile 0 lands first.
            x_tiles = []
            t0 = 0
            for ti, ts in enumerate(TILE_SIZES):
                x = xall.tile([P, 2, ts], f16, tag=f"x{ti}")
                eng = nc.sync if ti % 2 == 0 else nc.gpsimd
                eng.dma_start(
                    out=x[:, :, :], in_=x_ext[:, 2 * t0:2 * t0 + 2 * ts])
                x_tiles.append(x)
                t0 += ts

            # Phase B: compute + output stream
            t0 = 0
            for ti, ts in enumerate(TILE_SIZES):
                x = x_tiles[ti]

                # D[p,c,m,:ts] = Derivative_Erf(sqrt2*X - sqrt2*center_c),
                # one [P, 2*ts] op per center (fixed TMAX pitch, :ts slice)
                d = mid.tile([P, 3, 2, TMAX], f16, tag="d")
                for ci in range(3):
                    nc.scalar.activation(
                        d[:, ci, :, :ts],
                        x[:, :, :],
                        mybir.ActivationFunctionType.Derivative_Erf,
                        bias=bias_aps[ci][:, :],
                        scale=SQRT2,
                    )

                # ante[p,3j+k,:] = D[p,j,0,:] * D[p,k,1,:] -- 3 tensor_tensor
                # ops, j-plane broadcast via stride-0 middle dim. Exact-size
                # tile so each j-chunk is one contiguous per-partition run;
                # each chunk's DMA fires as soon as its product op is done,
                # smoothing the output stream. Buffers reused by size.
                ante = oute.tile([P, 9, ts], f16, tag=f"ante{ts}")
                dy = d[:, :, 1, :ts]
                for j in range(3):
                    dx_ap = bass.AP(
                        d.tensor, d.offset + j * 2 * TMAX,
                        [[6 * TMAX, P], [0, 3], [1, ts]],
                    )
                    nc.vector.tensor_tensor(
                        ante[:, 3 * j:3 * j + 3, :], dx_ap, dy,
                        op=mybir.AluOpType.mult,
                    )
                    eng = nc.sync if (3 * ti + j) % 2 == 0 else nc.scalar
                    eng.dma_start(
                        out=out_ext[:, 9 * t0 + 3 * j * ts:
                                    9 * t0 + 3 * (j + 1) * ts],
                        in_=ante[:, 3 * j:3 * j + 3, :])
                t0 += ts

    nc.compile()
    _nc_cache["nc"] = nc
    return nc


def _shard_host(feat2, src_shard, dst_shard):
    # [P, 2*R] tile-blocked per-edge coordinate deltas, fp16 on the wire
    g = (feat2[dst_shard] - feat2[src_shard]).astype(np.float16)  # [E_CORE, 2]
    g = g.reshape(P, R, 2)
    blocks = []
    t0 = 0
    for ts in TILE_SIZES:
        # [P, 2, ts]: plane-separated within the tile block
        blocks.append(g[:, t0:t0 + ts, :].transpose(0, 2, 1).reshape(P, 2 * ts))
        t0 += ts
    return np.ascontiguousarray(np.concatenate(blocks, axis=1))


def kernel(feat, edge_src, edge_dst, etypes):
    feat = np.asarray(feat, dtype=np.float32)
    edge_src = np.asarray(edge_src, dtype=np.int32)
    edge_dst = np.asarray(edge_dst, dtype=np.int32)
    del etypes  # unused by the reference computation

    nc = _build()

    feat2 = np.ascontiguousarray(feat[:, :2])  # only coords participate
    in_maps = []
    for c in range(N_CORES):
        sl = slice(c * E_CORE, (c + 1) * E_CORE)
        in_maps.append({
            "xy": _shard_host(feat2, edge_src[sl], edge_dst[sl]),
        })

    res = run_bass_kernel_spmd(nc, in_maps, core_ids=list(range(N_CORES)))
    out = np.empty((N_EDGES, 9), dtype=np.float32)
    scale = np.float32(PI_4)
    for c in range(N_CORES):
        r = res.results[c]["out"]          # [P, 9*R] fp16, (4/pi)*ante
        ov = out[c * E_CORE:(c + 1) * E_CORE].reshape(P, R, 9)
        t0 = 0
        for ts in TILE_SIZES:
            blk = r[:, 9 * t0:9 * t0 + 9 * ts].reshape(P, 9, ts)
            np.multiply(blk.transpose(0, 2, 1), scale, out=ov[:, t0:t0 + ts, :])
            t0 += ts
    return out
